# revision 26
# baseline (speedup 1.0000x reference)
"""Trainium2 Bass kernel for a 2-layer GRU model over ragged (packed) sequences.

Model (PyTorch gate order r,z,n):
  e   = LeakyReLU(x @ W1.T + b1, 0.01) @ W2.T + b2          # embedding MLP
  hs1 = GRU0(e),  hs2 = GRU1(hs1)      (masked: state held past seq end)
  out = LeakyReLU(h2_last, 0.01) @ fc_w.T + fc_b
  returns (out[B,8], h[2,B,96])

Distribution: data-parallel over batch, strided (core c gets b = c+8j).
`lengths` is sorted descending, so each core's live columns at step t are a
prefix; step t runs at width U[t] = ceil(K_{t-1}/8) and dead columns inside
the width are held exactly by injecting +BIG into the z-gate pre-activation
from a packed mask row (sigmoid(BIG) == 1.0 in fp32, so h' = h).

Layout: feature-major ([feature, batch]); no transposes on device. Biases
ride the matmuls via two ones-rows in the moving operand (hi/lo bf16 split).
All t-loop matmuls are bf16 at ragged width. The two GRU layers run skewed
(layer0@t with layer1@t-1) so every elementwise instruction covers both
layers at once, halving per-op overheads.
"""

import numpy as np
import ml_dtypes

import concourse.bacc as bacc
import concourse.mybir as mybir
from concourse.tile import TileContext
from concourse import bass_utils
from bass_rust import add_dep_helper

BF16 = ml_dtypes.bfloat16
F32 = np.float32

D_IN = 364
EMB1 = 128
H = 96
N_ACT = 8
NCORES = 8
SLOPE = 0.01
BIG = 30.0
DPAD = 384  # D_IN padded to 3*128; row D_IN carries ones for the b1 bias
PH1_COLS = 1024  # phase-1 outer DMA tile (columns)
PH1_MM = 512  # phase-1 matmul chunk (columns)
ALU = mybir.AluOpType
ACTF = mybir.ActivationFunctionType


def _gate(w, g):
    # rows of a [3H, *] torch-style stacked gate tensor; g in {0:r, 1:z, 2:n}
    return w[g * H:(g + 1) * H]


def _hilo(b):
    hi = b.astype(BF16).astype(F32)
    return hi, b - hi


def prep_host(x, lengths, h0, emb_w1, emb_b1, emb_w2, emb_b2,
              w_ih0, w_hh0, b_ih0, b_hh0, w_ih1, w_hh1, b_ih1, b_hh1,
              fc_w, fc_b):
    """Pack inputs host-side. Returns (meta, in_maps)."""
    x = np.asarray(x, F32)
    lengths = np.asarray(lengths).astype(np.int64)
    h0 = np.asarray(h0, F32)
    T, B, _ = x.shape
    NB = B // NCORES
    Tmax = int(lengths.max())

    # per-step widths: K_t live columns globally; step t is packed/processed
    # at width U[t] = W[t-1] (W non-increasing) so that merged ops for
    # (l0@t, l1@t-1) share one width
    K = (lengths[None, :] > np.arange(Tmax)[:, None]).sum(1)  # [Tmax]
    W = ((K + NCORES - 1) // NCORES).astype(np.int64)
    U = np.concatenate([W[:1], W[:-1]])  # U[t] = W[t-1], U[0] = W[0]
    OFF = np.zeros(Tmax + 1, np.int64)
    OFF[1:] = np.cumsum(U)
    NPACK = int(OFF[-1])

    # ---- shared (replicated) weight arrays ----
    emb_w1 = np.asarray(emb_w1, F32)
    emb_b1 = np.asarray(emb_b1, F32)
    emb_w2 = np.asarray(emb_w2, F32)
    emb_b2 = np.asarray(emb_b2, F32)
    fc_w = np.asarray(fc_w, F32)
    fc_b = np.asarray(fc_b, F32)
    w_ih = [np.asarray(w_ih0, F32), np.asarray(w_ih1, F32)]
    w_hh = [np.asarray(w_hh0, F32), np.asarray(w_hh1, F32)]
    b_ih = [np.asarray(b_ih0, F32), np.asarray(b_ih1, F32)]
    b_hh = [np.asarray(b_hh0, F32), np.asarray(b_hh1, F32)]

    # embedding layer 1, transposed + padded; row D_IN of the input carries
    # ones so the padded weight row holds b1
    ew1t = np.zeros((DPAD, EMB1), F32)
    ew1t[:D_IN] = emb_w1.T
    ew1t[D_IN] = emb_b1
    ew1t = ew1t.astype(BF16)
    ew2t = emb_w2.T.astype(BF16)  # [128, 96]

    # b2 folds into layer-0 x-side biases; r,z h-side biases fold into the
    # x side too (n keeps b_hh on the h side: r multiplies it separately)
    bx = [b_ih[0] + w_ih[0] @ emb_b2, b_ih[1].copy()]
    for l in range(2):
        bx[l][0:2 * H] += b_hh[l][0:2 * H]

    # gate lhsT blocks [98, 96]: rows 0:96 = W.T, rows 96/97 = bias hi/lo
    def gate_block(wmat, bias, g):
        blk = np.zeros((H + 2, H), F32)
        blk[:H] = _gate(wmat, g).T
        blk[H], blk[H + 1] = _hilo(_gate(bias, g))
        return blk

    wx = np.zeros((2, 3, H + 2, H), F32)
    wh = np.zeros((2, 3, H + 2, H), F32)
    zero_b = np.zeros(3 * H, F32)
    for l in range(2):
        for g in range(3):
            wx[l, g] = gate_block(w_ih[l], bx[l], g)
            wh[l, g] = gate_block(w_hh[l], b_hh[l] if g == 2 else zero_b, g)
    wx = wx.astype(BF16)
    wh = wh.astype(BF16)

    mz = np.full((1, H), BIG, F32).astype(BF16)  # dead-column z injection

    fct = np.zeros((H + 1, N_ACT), F32)  # fc head stays fp32 (one matmul)
    fct[:H] = fc_w.T
    fct[H] = fc_b

    shared = dict(ew1t=ew1t, ew2t=ew2t, wx=wx, wh=wh, mz=mz, fct=fct)

    # ---- per-core packed inputs ----
    in_maps = []
    for c in range(NCORES):
        xc = x[:, c::NCORES, :]  # [T, NB, D] view
        lens_c = lengths[c::NCORES]  # sorted desc
        cols = [xc[t, :U[t], :] for t in range(Tmax)]
        xcat = np.concatenate(cols, axis=0) if cols else np.zeros((0, D_IN), F32)
        xp = np.zeros((DPAD, NPACK), BF16)
        xp[:D_IN] = np.ascontiguousarray(xcat.T).astype(BF16)
        xp[D_IN] = BF16(1.0)
        mp = np.zeros((3, NPACK), BF16)
        mp[0] = BF16(1.0)
        mp[1] = BF16(1.0)
        dead = (np.concatenate([(lens_c[:U[t]] <= t) for t in range(Tmax)])
                if Tmax else np.zeros(0, bool))
        mp[2] = dead.astype(BF16)
        h0t = np.ascontiguousarray(
            h0[:, c::NCORES, :].transpose(0, 2, 1)).astype(BF16)  # [2, 96, NB]
        ones_nb = np.ones((2, NB), BF16)
        in_maps.append(dict(xp=xp, mp=mp, h0t=h0t, ones_nb=ones_nb, **shared))

    meta = dict(T=T, B=B, NB=NB, Tmax=Tmax, NPACK=NPACK,
                U=U.tolist(), OFF=OFF.tolist())
    return meta, in_maps


def build_program(meta):
    NB = meta["NB"]
    Tmax = meta["Tmax"]
    NPACK = meta["NPACK"]
    U = meta["U"]
    OFF = meta["OFF"]
    f32 = mybir.dt.float32
    bf16 = mybir.dt.bfloat16

    nc = bacc.Bacc(trn_type="TRN2", target_bir_lowering=False, debug=False,
                   num_devices=NCORES)

    # DRAM I/O
    d_xp = nc.dram_tensor("xp", [DPAD, NPACK], bf16, kind="ExternalInput").ap()
    d_mp = nc.dram_tensor("mp", [3, NPACK], bf16, kind="ExternalInput").ap()
    d_h0t = nc.dram_tensor("h0t", [2, H, NB], bf16, kind="ExternalInput").ap()
    d_ones = nc.dram_tensor("ones_nb", [2, NB], bf16, kind="ExternalInput").ap()
    d_ew1t = nc.dram_tensor("ew1t", [DPAD, EMB1], bf16, kind="ExternalInput").ap()
    d_ew2t = nc.dram_tensor("ew2t", [EMB1, H], bf16, kind="ExternalInput").ap()
    d_wx = nc.dram_tensor("wx", [2, 3, H + 2, H], bf16, kind="ExternalInput").ap()
    d_wh = nc.dram_tensor("wh", [2, 3, H + 2, H], bf16, kind="ExternalInput").ap()
    d_mz = nc.dram_tensor("mz", [1, H], bf16, kind="ExternalInput").ap()
    d_fct = nc.dram_tensor("fct", [H + 1, N_ACT], f32, kind="ExternalInput").ap()
    d_outT = nc.dram_tensor("outT", [N_ACT, NB], f32, kind="ExternalOutput").ap()
    d_hlast = nc.dram_tensor("hlast", [2, H, NB], f32, kind="ExternalOutput").ap()

    with TileContext(nc) as tc:
        with (
            tc.tile_pool(name="persist", bufs=1) as persist,
            tc.tile_pool(name="xin", bufs=2) as xin,
            tc.tile_pool(name="a1p", bufs=3) as a1p,
            tc.tile_pool(name="gtmp", bufs=3) as gtmp,
            tc.tile_pool(name="psum", bufs=2, space="PSUM") as psp,
        ):
            # ---- persistent tiles ----
            E = persist.tile([H + 2, NPACK], bf16)  # packed e + 2 ones rows
            M1 = persist.tile([1, NPACK], bf16)  # dead-column mask row
            nc.sync.dma_start(out=E[H:H + 2, :], in_=d_mp[0:2, :])
            nc.sync.dma_start(out=M1, in_=d_mp[2:3, :])
            EW1 = persist.tile([128, 3, EMB1], bf16)
            for ci in range(3):
                nc.sync.dma_start(out=EW1[:, ci, :],
                                  in_=d_ew1t[ci * 128:(ci + 1) * 128, :])
            EW2 = persist.tile([EMB1, H], bf16)
            nc.sync.dma_start(out=EW2, in_=d_ew2t)
            WX = persist.tile([H + 2, 2, 3, H], bf16)
            WHT = persist.tile([H + 2, 2, 3, H], bf16)
            for l in range(2):
                for g in range(3):
                    nc.sync.dma_start(out=WX[:, l, g, :], in_=d_wx[l, g])
                    nc.sync.dma_start(out=WHT[:, l, g, :], in_=d_wh[l, g])
            MZ = persist.tile([1, H], bf16)
            nc.sync.dma_start(out=MZ, in_=d_mz)
            FCT = persist.tile([H + 1, N_ACT], f32)
            nc.sync.dma_start(out=FCT, in_=d_fct)
            # HH: rows 0:96 = h (dim1: layer), rows 96/97 = ones
            HH = persist.tile([H + 2, 2, NB], bf16)
            for l in range(2):
                nc.sync.dma_start(out=HH[0:H, l, :], in_=d_h0t[l])
                nc.sync.dma_start(out=HH[H:H + 2, l, :], in_=d_ones)

            # ---- phase 1: embedding MLP over all packed columns ----
            for c0 in range(0, NPACK, PH1_COLS):
                c1 = min(NPACK, c0 + PH1_COLS)
                w = c1 - c0
                xt = xin.tile([128, 3, PH1_COLS], bf16, tag="xt")
                for ci in range(3):
                    nc.sync.dma_start(out=xt[:, ci, 0:w],
                                      in_=d_xp[ci * 128:(ci + 1) * 128, c0:c1])
                for j0 in range(0, w, PH1_MM):
                    j1 = min(w, j0 + PH1_MM)
                    jw = j1 - j0
                    pa = psp.tile([128, PH1_MM], f32, tag="psA")
                    for ci in range(3):
                        nc.tensor.matmul(pa[:, 0:jw], EW1[:, ci, :],
                                         xt[:, ci, j0:j1],
                                         start=(ci == 0), stop=(ci == 2))
                    a0 = a1p.tile([128, PH1_MM], f32, tag="a0")
                    nc.scalar.copy(a0[:, 0:jw], pa[:, 0:jw])
                    a1 = a1p.tile([128, PH1_MM], bf16, tag="a1")
                    # LeakyReLU: max(v, 0.01*v); b1 already in psum (ones row)
                    nc.vector.scalar_tensor_tensor(
                        out=a1[:, 0:jw], in0=a0[:, 0:jw], scalar=SLOPE,
                        in1=a0[:, 0:jw], op0=ALU.mult, op1=ALU.max)
                    pb = psp.tile([H, PH1_MM], f32, tag="psB")
                    nc.tensor.matmul(pb[:, 0:jw], EW2, a1[:, 0:jw],
                                     start=True, stop=True)
                    nc.scalar.copy(E[0:H, c0 + j0:c0 + j1], pb[:, 0:jw])

            # ---- phase 2: skewed fused GRU steps ----
            # merged step s computes l0@s (if s<Tmax) + l1@s-1 (if s>0),
            # both at width U[s] (U[s] <= U[s-1], so l1 ops are in-range).
            # PSUM layouts keep every psum-reading instruction in one bank:
            #   psRZ: bank A = [r0, z0], bank B = [r1, z1]
            #   psN:  bank A = [nh0, nh1], bank B = [nx0, nx1]
            # SBUF RZ layout: [r0, r1, z0, z1] so r/z pairs are contiguous.
            for s in range(Tmax + 1):
                u = U[s] if s < Tmax else U[Tmax - 1]
                off = OFF[s] if s < Tmax else None
                off1 = OFF[s - 1] if s > 0 else None
                do0 = s < Tmax
                do1 = s > 0
                la = 0 if do0 else 1
                lb = 2 if do1 else 1

                psRZ = psp.tile([H, 4, NB], f32, tag="psA")
                psN = psp.tile([H, 4, NB], f32, tag="psB")
                # One accumulation group per psum bank (start=True marks the
                # whole 2KB zero-region, so banks can't host two live groups)
                # with explicit ordering deps since Tile won't order writes
                # to disjoint regions. x-side matmuls first (off the h-chain),
                # h-side last; r/z h-matmuls before n so sigmoids unblock.
                bankRZ0, bankRZ1, bankNH, bankNX = [], [], [], []
                if do0:
                    xr = E[0:H + 2, off:off + u]
                    hr = HH[0:H + 2, 0, 0:u]
                    bankRZ0 += [
                        (psRZ[:, 0, 0:u], WX[:, 0, 0, :], xr),
                        (psRZ[:, 1, 0:u], WX[:, 0, 1, :], xr),
                        (psRZ[:, 1, 0:u], MZ, M1[0:1, off:off + u]),
                        (psRZ[:, 0, 0:u], WHT[:, 0, 0, :], hr),
                        (psRZ[:, 1, 0:u], WHT[:, 0, 1, :], hr),
                    ]
                    bankNX.append((psN[:, 2, 0:u], WX[:, 0, 2, :], xr))
                    bankNH.append((psN[:, 0, 0:u], WHT[:, 0, 2, :], hr))
                if do1:
                    xr1 = HH[0:H + 2, 0, 0:u]  # h0 before this step's update
                    hr1 = HH[0:H + 2, 1, 0:u]
                    bankRZ1 += [
                        (psRZ[:, 2, 0:u], WX[:, 1, 0, :], xr1),
                        (psRZ[:, 3, 0:u], WX[:, 1, 1, :], xr1),
                        (psRZ[:, 3, 0:u], MZ, M1[0:1, off1:off1 + u]),
                        (psRZ[:, 2, 0:u], WHT[:, 1, 0, :], hr1),
                        (psRZ[:, 3, 0:u], WHT[:, 1, 1, :], hr1),
                    ]
                    bankNX.append((psN[:, 3, 0:u], WX[:, 1, 2, :], xr1))
                    bankNH.append((psN[:, 1, 0:u], WHT[:, 1, 2, :], hr1))
                # emission: strictly in-bank order (bisect: h-side kept last
                # inside each bank but banks emitted whole)
                order = []
                for bank in (bankRZ0, bankRZ1, bankNX, bankNH):
                    for i, mm in enumerate(bank):
                        order.append((bank, i, mm))
                prev = {}
                for bank, i, (o, lhs, rhs) in order:
                    inst = nc.tensor.matmul(o, lhs, rhs, start=(i == 0),
                                            stop=(i == len(bank) - 1),
                                            skip_group_check=True)
                    key = id(bank)
                    if key in prev:
                        add_dep_helper(inst.ins, prev[key].ins, False,
                                       "psum bank group order")
                    prev[key] = inst

                RZ = gtmp.tile([H, 4, NB], bf16, tag="RZ")
                for li in range(la, lb):
                    # psRZ bank for layer li -> RZ positions {li, 2+li}
                    nc.scalar.activation(RZ[:, li:li + 3:2, 0:u],
                                         psRZ[:, 2 * li:2 * li + 2, 0:u],
                                         ACTF.Sigmoid)
                T2 = gtmp.tile([H, 2, NB], f32, tag="T2")
                nc.vector.tensor_tensor(
                    out=T2[:, la:lb, 0:u], in0=psN[:, la:lb, 0:u],
                    in1=RZ[:, la:lb, 0:u], op=ALU.mult)
                PRE = gtmp.tile([H, 2, NB], f32, tag="PRE")
                nc.vector.tensor_tensor(
                    out=PRE[:, la:lb, 0:u], in0=T2[:, la:lb, 0:u],
                    in1=psN[:, 2 + la:2 + lb, 0:u], op=ALU.add)
                NT = gtmp.tile([H, 2, NB], bf16, tag="NT")
                nc.scalar.activation(NT[:, la:lb, 0:u], PRE[:, la:lb, 0:u],
                                     ACTF.Tanh)
                WT = gtmp.tile([H, 2, NB], bf16, tag="WT")
                nc.vector.scalar_tensor_tensor(
                    out=WT[:, la:lb, 0:u], in0=NT[:, la:lb, 0:u], scalar=-1.0,
                    in1=HH[0:H, la:lb, 0:u], op0=ALU.mult, op1=ALU.add)
                VT = gtmp.tile([H, 2, NB], bf16, tag="VT")
                nc.vector.tensor_tensor(
                    out=VT[:, la:lb, 0:u], in0=RZ[:, 2 + la:2 + lb, 0:u],
                    in1=WT[:, la:lb, 0:u], op=ALU.mult)
                nc.vector.tensor_tensor(
                    out=HH[0:H, la:lb, 0:u], in0=NT[:, la:lb, 0:u],
                    in1=VT[:, la:lb, 0:u], op=ALU.add)

            # ---- fc head + outputs ----
            HL = persist.tile([H, 2, NB], f32)
            nc.scalar.copy(HL[:, 0, :], HH[0:H, 0, :])
            nc.scalar.copy(HL[:, 1, :], HH[0:H, 1, :])
            nc.sync.dma_start(out=d_hlast[0], in_=HL[:, 0, :])
            nc.sync.dma_start(out=d_hlast[1], in_=HL[:, 1, :])
            LH = persist.tile([H + 1, NB], f32)
            nc.vector.memset(LH[H:H + 1, :], 1.0)
            nc.vector.scalar_tensor_tensor(
                out=LH[0:H, :], in0=HL[:, 1, :], scalar=SLOPE,
                in1=HL[:, 1, :], op0=ALU.mult, op1=ALU.max)
            psO = psp.tile([N_ACT, NB], f32, tag="psB")
            nc.tensor.matmul(psO, FCT, LH, start=True, stop=True)
            OS = persist.tile([N_ACT, NB], f32)
            nc.vector.tensor_copy(OS, psO)
            nc.sync.dma_start(out=d_outT, in_=OS)

    nc.compile()
    return nc


def kernel(**inputs):
    meta, in_maps = prep_host(**inputs)
    nc = build_program(meta)
    res = bass_utils.run_bass_kernel_spmd(nc, in_maps,
                                          core_ids=list(range(NCORES)))
    B = meta["B"]
    out = np.zeros((B, N_ACT), F32)
    h = np.zeros((2, B, H), F32)
    for c in range(NCORES):
        r = res.results[c]
        out[c::NCORES, :] = r["outT"].T
        h[0, c::NCORES, :] = r["hlast"][0].T
        h[1, c::NCORES, :] = r["hlast"][1].T
    return out, h


# revision 27
# speedup vs baseline: 1.0530x; 1.0530x over previous
"""Trainium2 Bass kernel for a 2-layer GRU model over ragged (packed) sequences.

Model (PyTorch gate order r,z,n):
  e   = LeakyReLU(x @ W1.T + b1, 0.01) @ W2.T + b2          # embedding MLP
  hs1 = GRU0(e),  hs2 = GRU1(hs1)      (masked: state held past seq end)
  out = LeakyReLU(h2_last, 0.01) @ fc_w.T + fc_b
  returns (out[B,8], h[2,B,96])

Distribution: data-parallel over batch, strided (core c gets b = c+8j).
`lengths` is sorted descending, so each core's live columns at step t are a
prefix; step t runs at width U[t] = ceil(K_{t-1}/8) and dead columns inside
the width are held exactly by injecting +BIG into the z-gate pre-activation
from a packed mask row (sigmoid(BIG) == 1.0 in fp32, so h' = h).

Layout: feature-major ([feature, batch]); no transposes on device. Biases
ride the matmuls via two ones-rows in the moving operand (hi/lo bf16 split).
All t-loop matmuls are bf16 at ragged width. The two GRU layers run skewed
(layer0@t with layer1@t-1) so every elementwise instruction covers both
layers at once, halving per-op overheads.
"""

import numpy as np
import ml_dtypes

import concourse.bacc as bacc
import concourse.mybir as mybir
from concourse.tile import TileContext
from concourse import bass_utils
from bass_rust import add_dep_helper

BF16 = ml_dtypes.bfloat16
F32 = np.float32

D_IN = 364
EMB1 = 128
H = 96
N_ACT = 8
NCORES = 8
SLOPE = 0.01
BIG = 30.0
DPAD = 384  # D_IN padded to 3*128; row D_IN carries ones for the b1 bias
PH1_COLS = 1024  # phase-1 outer DMA tile (columns)
PH1_MM = 512  # phase-1 matmul chunk (columns)
ALU = mybir.AluOpType
ACTF = mybir.ActivationFunctionType


def _gate(w, g):
    # rows of a [3H, *] torch-style stacked gate tensor; g in {0:r, 1:z, 2:n}
    return w[g * H:(g + 1) * H]


def _hilo(b):
    hi = b.astype(BF16).astype(F32)
    return hi, b - hi


def prep_host(x, lengths, h0, emb_w1, emb_b1, emb_w2, emb_b2,
              w_ih0, w_hh0, b_ih0, b_hh0, w_ih1, w_hh1, b_ih1, b_hh1,
              fc_w, fc_b):
    """Pack inputs host-side. Returns (meta, in_maps)."""
    x = np.asarray(x, F32)
    lengths = np.asarray(lengths).astype(np.int64)
    h0 = np.asarray(h0, F32)
    T, B, _ = x.shape
    NB = B // NCORES
    Tmax = int(lengths.max())

    # per-step widths: K_t live columns globally; step t is packed/processed
    # at width U[t] = W[t-1] (W non-increasing) so that merged ops for
    # (l0@t, l1@t-1) share one width
    K = (lengths[None, :] > np.arange(Tmax)[:, None]).sum(1)  # [Tmax]
    W = ((K + NCORES - 1) // NCORES).astype(np.int64)
    U = np.concatenate([W[:1], W[:-1]])  # U[t] = W[t-1], U[0] = W[0]
    OFF = np.zeros(Tmax + 1, np.int64)
    OFF[1:] = np.cumsum(U)
    NPACK = int(OFF[-1])

    # ---- shared (replicated) weight arrays ----
    emb_w1 = np.asarray(emb_w1, F32)
    emb_b1 = np.asarray(emb_b1, F32)
    emb_w2 = np.asarray(emb_w2, F32)
    emb_b2 = np.asarray(emb_b2, F32)
    fc_w = np.asarray(fc_w, F32)
    fc_b = np.asarray(fc_b, F32)
    w_ih = [np.asarray(w_ih0, F32), np.asarray(w_ih1, F32)]
    w_hh = [np.asarray(w_hh0, F32), np.asarray(w_hh1, F32)]
    b_ih = [np.asarray(b_ih0, F32), np.asarray(b_ih1, F32)]
    b_hh = [np.asarray(b_hh0, F32), np.asarray(b_hh1, F32)]

    # embedding layer 1, transposed + padded; row D_IN of the input carries
    # ones so the padded weight row holds b1
    ew1t = np.zeros((DPAD, EMB1), F32)
    ew1t[:D_IN] = emb_w1.T
    ew1t[D_IN] = emb_b1
    ew1t = ew1t.astype(BF16)
    ew2t = emb_w2.T.astype(BF16)  # [128, 96]

    # b2 folds into layer-0 x-side biases; r,z h-side biases fold into the
    # x side too (n keeps b_hh on the h side: r multiplies it separately)
    bx = [b_ih[0] + w_ih[0] @ emb_b2, b_ih[1].copy()]
    for l in range(2):
        bx[l][0:2 * H] += b_hh[l][0:2 * H]

    # gate lhsT blocks [98, 96]: rows 0:96 = W.T, rows 96/97 = bias hi/lo
    def gate_block(wmat, bias, g):
        blk = np.zeros((H + 2, H), F32)
        blk[:H] = _gate(wmat, g).T
        blk[H], blk[H + 1] = _hilo(_gate(bias, g))
        return blk

    wx = np.zeros((2, 3, H + 2, H), F32)
    wh = np.zeros((2, 3, H + 2, H), F32)
    zero_b = np.zeros(3 * H, F32)
    for l in range(2):
        for g in range(3):
            wx[l, g] = gate_block(w_ih[l], bx[l], g)
            wh[l, g] = gate_block(w_hh[l], b_hh[l] if g == 2 else zero_b, g)
    wx = wx.astype(BF16)
    wh = wh.astype(BF16)

    mz = np.full((1, H), BIG, F32).astype(BF16)  # dead-column z injection

    fct = np.zeros((H + 1, N_ACT), F32)  # fc head stays fp32 (one matmul)
    fct[:H] = fc_w.T
    fct[H] = fc_b

    shared = dict(ew1t=ew1t, ew2t=ew2t, wx=wx, wh=wh, mz=mz, fct=fct)

    # ---- per-core packed inputs ----
    in_maps = []
    for c in range(NCORES):
        xc = x[:, c::NCORES, :]  # [T, NB, D] view
        lens_c = lengths[c::NCORES]  # sorted desc
        cols = [xc[t, :U[t], :] for t in range(Tmax)]
        xcat = np.concatenate(cols, axis=0) if cols else np.zeros((0, D_IN), F32)
        xp = np.zeros((DPAD, NPACK), BF16)
        xp[:D_IN] = np.ascontiguousarray(xcat.T).astype(BF16)
        xp[D_IN] = BF16(1.0)
        mp = np.zeros((3, NPACK), BF16)
        mp[0] = BF16(1.0)
        mp[1] = BF16(1.0)
        dead = (np.concatenate([(lens_c[:U[t]] <= t) for t in range(Tmax)])
                if Tmax else np.zeros(0, bool))
        mp[2] = dead.astype(BF16)
        h0t = np.ascontiguousarray(
            h0[:, c::NCORES, :].transpose(0, 2, 1)).astype(BF16)  # [2, 96, NB]
        ones_nb = np.ones((2, NB), BF16)
        in_maps.append(dict(xp=xp, mp=mp, h0t=h0t, ones_nb=ones_nb, **shared))

    meta = dict(T=T, B=B, NB=NB, Tmax=Tmax, NPACK=NPACK,
                U=U.tolist(), OFF=OFF.tolist())
    return meta, in_maps


def build_program(meta):
    NB = meta["NB"]
    Tmax = meta["Tmax"]
    NPACK = meta["NPACK"]
    U = meta["U"]
    OFF = meta["OFF"]
    f32 = mybir.dt.float32
    bf16 = mybir.dt.bfloat16

    nc = bacc.Bacc(trn_type="TRN2", target_bir_lowering=False, debug=False,
                   num_devices=NCORES)

    # DRAM I/O
    d_xp = nc.dram_tensor("xp", [DPAD, NPACK], bf16, kind="ExternalInput").ap()
    d_mp = nc.dram_tensor("mp", [3, NPACK], bf16, kind="ExternalInput").ap()
    d_h0t = nc.dram_tensor("h0t", [2, H, NB], bf16, kind="ExternalInput").ap()
    d_ones = nc.dram_tensor("ones_nb", [2, NB], bf16, kind="ExternalInput").ap()
    d_ew1t = nc.dram_tensor("ew1t", [DPAD, EMB1], bf16, kind="ExternalInput").ap()
    d_ew2t = nc.dram_tensor("ew2t", [EMB1, H], bf16, kind="ExternalInput").ap()
    d_wx = nc.dram_tensor("wx", [2, 3, H + 2, H], bf16, kind="ExternalInput").ap()
    d_wh = nc.dram_tensor("wh", [2, 3, H + 2, H], bf16, kind="ExternalInput").ap()
    d_mz = nc.dram_tensor("mz", [1, H], bf16, kind="ExternalInput").ap()
    d_fct = nc.dram_tensor("fct", [H + 1, N_ACT], f32, kind="ExternalInput").ap()
    d_outT = nc.dram_tensor("outT", [N_ACT, NB], f32, kind="ExternalOutput").ap()
    d_hlast = nc.dram_tensor("hlast", [2, H, NB], f32, kind="ExternalOutput").ap()

    with TileContext(nc) as tc:
        with (
            tc.tile_pool(name="persist", bufs=1) as persist,
            tc.tile_pool(name="xin", bufs=2) as xin,
            tc.tile_pool(name="a1p", bufs=3) as a1p,
            tc.tile_pool(name="gtmp", bufs=3) as gtmp,
            tc.tile_pool(name="psum", bufs=2, space="PSUM") as psp,
        ):
            # ---- persistent tiles ----
            E = persist.tile([H + 2, NPACK], bf16)  # packed e + 2 ones rows
            M1 = persist.tile([1, NPACK], bf16)  # dead-column mask row
            nc.sync.dma_start(out=E[H:H + 2, :], in_=d_mp[0:2, :])
            nc.sync.dma_start(out=M1, in_=d_mp[2:3, :])
            EW1 = persist.tile([128, 3, EMB1], bf16)
            for ci in range(3):
                nc.sync.dma_start(out=EW1[:, ci, :],
                                  in_=d_ew1t[ci * 128:(ci + 1) * 128, :])
            EW2 = persist.tile([EMB1, H], bf16)
            nc.sync.dma_start(out=EW2, in_=d_ew2t)
            WX = persist.tile([H + 2, 2, 3, H], bf16)
            WHT = persist.tile([H + 2, 2, 3, H], bf16)
            for l in range(2):
                for g in range(3):
                    nc.sync.dma_start(out=WX[:, l, g, :], in_=d_wx[l, g])
                    nc.sync.dma_start(out=WHT[:, l, g, :], in_=d_wh[l, g])
            MZ = persist.tile([1, H], bf16)
            nc.sync.dma_start(out=MZ, in_=d_mz)
            FCT = persist.tile([H + 1, N_ACT], f32)
            nc.sync.dma_start(out=FCT, in_=d_fct)
            # HH: rows 0:96 = h (dim1: layer), rows 96/97 = ones
            HH = persist.tile([H + 2, 2, NB], bf16)
            for l in range(2):
                nc.sync.dma_start(out=HH[0:H, l, :], in_=d_h0t[l])
                nc.sync.dma_start(out=HH[H:H + 2, l, :], in_=d_ones)

            # ---- phase 1: embedding MLP over all packed columns ----
            for c0 in range(0, NPACK, PH1_COLS):
                c1 = min(NPACK, c0 + PH1_COLS)
                w = c1 - c0
                xt = xin.tile([128, 3, PH1_COLS], bf16, tag="xt")
                for ci in range(3):
                    nc.sync.dma_start(out=xt[:, ci, 0:w],
                                      in_=d_xp[ci * 128:(ci + 1) * 128, c0:c1])
                for j0 in range(0, w, PH1_MM):
                    j1 = min(w, j0 + PH1_MM)
                    jw = j1 - j0
                    pa = psp.tile([128, PH1_MM], f32, tag="psA")
                    for ci in range(3):
                        nc.tensor.matmul(pa[:, 0:jw], EW1[:, ci, :],
                                         xt[:, ci, j0:j1],
                                         start=(ci == 0), stop=(ci == 2))
                    a0 = a1p.tile([128, PH1_MM], f32, tag="a0")
                    nc.scalar.copy(a0[:, 0:jw], pa[:, 0:jw])
                    a1 = a1p.tile([128, PH1_MM], bf16, tag="a1")
                    # LeakyReLU: max(v, 0.01*v); b1 already in psum (ones row)
                    nc.vector.scalar_tensor_tensor(
                        out=a1[:, 0:jw], in0=a0[:, 0:jw], scalar=SLOPE,
                        in1=a0[:, 0:jw], op0=ALU.mult, op1=ALU.max)
                    pb = psp.tile([H, PH1_MM], f32, tag="psB")
                    nc.tensor.matmul(pb[:, 0:jw], EW2, a1[:, 0:jw],
                                     start=True, stop=True)
                    nc.scalar.copy(E[0:H, c0 + j0:c0 + j1], pb[:, 0:jw])

            # ---- phase 2: skewed fused GRU steps ----
            # merged step s computes l0@s (if s<Tmax) + l1@s-1 (if s>0),
            # both at width U[s] (U[s] <= U[s-1], so l1 ops are in-range).
            # PSUM layouts keep every psum-reading instruction in one bank:
            #   psRZ: bank A = [r0, z0], bank B = [r1, z1]
            #   psN:  bank A = [nh0, nh1], bank B = [nx0, nx1]
            # SBUF RZ layout: [r0, r1, z0, z1] so r/z pairs are contiguous.
            for s in range(Tmax + 1):
                u = U[s] if s < Tmax else U[Tmax - 1]
                off = OFF[s] if s < Tmax else None
                off1 = OFF[s - 1] if s > 0 else None
                do0 = s < Tmax
                do1 = s > 0
                la = 0 if do0 else 1
                lb = 2 if do1 else 1

                small = u <= 128
                # For u<=128 all four r/z regions fit one 2KB psum bank: one
                # accumulation group and ONE merged sigmoid. Otherwise two
                # banks / two sigmoids. psRZ layout: [r0, z0, r1, z1];
                # psN: [nh0, nh1, nx0, nx1]; RZ (sbuf) mirrors psRZ order.
                uu = 128 if small else NB
                psRZ = psp.tile([H, 4, uu], f32, tag="psA")
                psN = psp.tile([H, 4, uu], f32, tag="psB")
                rz0, rz1, bnh, bnx = [], [], [], []
                if do0:
                    xr = E[0:H + 2, off:off + u]
                    hr = HH[0:H + 2, 0, 0:u]
                    rz0 += [
                        (psRZ[:, 0, 0:u], WX[:, 0, 0, :], xr),
                        (psRZ[:, 1, 0:u], WX[:, 0, 1, :], xr),
                        (psRZ[:, 1, 0:u], MZ, M1[0:1, off:off + u]),
                    ]
                    bnx.append((psN[:, 2, 0:u], WX[:, 0, 2, :], xr))
                if do1:
                    xr1 = HH[0:H + 2, 0, 0:u]  # h0 before this step's update
                    rz1 += [
                        (psRZ[:, 2, 0:u], WX[:, 1, 0, :], xr1),
                        (psRZ[:, 3, 0:u], WX[:, 1, 1, :], xr1),
                        (psRZ[:, 3, 0:u], MZ, M1[0:1, off1:off1 + u]),
                    ]
                    bnx.append((psN[:, 3, 0:u], WX[:, 1, 2, :], xr1))
                # h-side (on the chain): r/z first so sigmoids unblock, n last
                if do0:
                    hr = HH[0:H + 2, 0, 0:u]
                    rz0 += [(psRZ[:, 0, 0:u], WHT[:, 0, 0, :], hr),
                            (psRZ[:, 1, 0:u], WHT[:, 0, 1, :], hr)]
                    bnh.append((psN[:, 0, 0:u], WHT[:, 0, 2, :], hr))
                if do1:
                    hr1 = HH[0:H + 2, 1, 0:u]
                    rz1 += [(psRZ[:, 2, 0:u], WHT[:, 1, 0, :], hr1),
                            (psRZ[:, 3, 0:u], WHT[:, 1, 1, :], hr1)]
                    bnh.append((psN[:, 1, 0:u], WHT[:, 1, 2, :], hr1))
                if small:
                    banks = [rz0 + rz1, bnx + bnh]
                else:
                    banks = [rz0, rz1, bnx, bnh]
                for bank in banks:
                    prev = None
                    for i, (o, lhs, rhs) in enumerate(bank):
                        inst = nc.tensor.matmul(o, lhs, rhs, start=(i == 0),
                                                stop=(i == len(bank) - 1),
                                                skip_group_check=True)
                        if prev is not None:
                            add_dep_helper(inst.ins, prev.ins, False,
                                           "psum bank group order")
                        prev = inst

                RZ = gtmp.tile([H, 4, NB], bf16, tag="RZ")  # [r0,z0,r1,z1]
                if small and do0 and do1:
                    nc.scalar.activation(RZ[:, 0:4, 0:u], psRZ[:, 0:4, 0:u],
                                         ACTF.Sigmoid)
                else:
                    for li in range(la, lb):
                        nc.scalar.activation(RZ[:, 2 * li:2 * li + 2, 0:u],
                                             psRZ[:, 2 * li:2 * li + 2, 0:u],
                                             ACTF.Sigmoid)
                T2 = gtmp.tile([H, 2, NB], f32, tag="T2")
                nc.vector.tensor_tensor(
                    out=T2[:, la:lb, 0:u], in0=psN[:, la:lb, 0:u],
                    in1=RZ[:, 2 * la:2 * lb:2, 0:u], op=ALU.mult)
                PRE = gtmp.tile([H, 2, NB], f32, tag="PRE")
                nc.vector.tensor_tensor(
                    out=PRE[:, la:lb, 0:u], in0=T2[:, la:lb, 0:u],
                    in1=psN[:, 2 + la:2 + lb, 0:u], op=ALU.add)
                NT = gtmp.tile([H, 2, NB], bf16, tag="NT")
                nc.scalar.activation(NT[:, la:lb, 0:u], PRE[:, la:lb, 0:u],
                                     ACTF.Tanh)
                WT = gtmp.tile([H, 2, NB], bf16, tag="WT")
                nc.vector.tensor_tensor(
                    out=WT[:, la:lb, 0:u], in0=HH[0:H, la:lb, 0:u],
                    in1=NT[:, la:lb, 0:u], op=ALU.subtract)
                VT = gtmp.tile([H, 2, NB], bf16, tag="VT")
                nc.vector.tensor_tensor(
                    out=VT[:, la:lb, 0:u], in0=RZ[:, 2 * la + 1:2 * lb:2, 0:u],
                    in1=WT[:, la:lb, 0:u], op=ALU.mult)
                nc.vector.tensor_tensor(
                    out=HH[0:H, la:lb, 0:u], in0=NT[:, la:lb, 0:u],
                    in1=VT[:, la:lb, 0:u], op=ALU.add)

            # ---- fc head + outputs ----
            HL = persist.tile([H, 2, NB], f32)
            nc.scalar.copy(HL[:, 0, :], HH[0:H, 0, :])
            nc.scalar.copy(HL[:, 1, :], HH[0:H, 1, :])
            nc.sync.dma_start(out=d_hlast[0], in_=HL[:, 0, :])
            nc.sync.dma_start(out=d_hlast[1], in_=HL[:, 1, :])
            LH = persist.tile([H + 1, NB], f32)
            nc.vector.memset(LH[H:H + 1, :], 1.0)
            nc.vector.scalar_tensor_tensor(
                out=LH[0:H, :], in0=HL[:, 1, :], scalar=SLOPE,
                in1=HL[:, 1, :], op0=ALU.mult, op1=ALU.max)
            psO = psp.tile([N_ACT, NB], f32, tag="psB")
            nc.tensor.matmul(psO, FCT, LH, start=True, stop=True)
            OS = persist.tile([N_ACT, NB], f32)
            nc.vector.tensor_copy(OS, psO)
            nc.sync.dma_start(out=d_outT, in_=OS)

    nc.compile()
    return nc


def kernel(**inputs):
    meta, in_maps = prep_host(**inputs)
    nc = build_program(meta)
    res = bass_utils.run_bass_kernel_spmd(nc, in_maps,
                                          core_ids=list(range(NCORES)))
    B = meta["B"]
    out = np.zeros((B, N_ACT), F32)
    h = np.zeros((2, B, H), F32)
    for c in range(NCORES):
        r = res.results[c]
        out[c::NCORES, :] = r["outT"].T
        h[0, c::NCORES, :] = r["hlast"][0].T
        h[1, c::NCORES, :] = r["hlast"][1].T
    return out, h


# revision 28
# speedup vs baseline: 1.0766x; 1.0224x over previous
"""Trainium2 Bass kernel for a 2-layer GRU model over ragged (packed) sequences.

Model (PyTorch gate order r,z,n):
  e   = LeakyReLU(x @ W1.T + b1, 0.01) @ W2.T + b2          # embedding MLP
  hs1 = GRU0(e),  hs2 = GRU1(hs1)      (masked: state held past seq end)
  out = LeakyReLU(h2_last, 0.01) @ fc_w.T + fc_b
  returns (out[B,8], h[2,B,96])

Distribution: data-parallel over batch, strided (core c gets b = c+8j).
`lengths` is sorted descending, so each core's live columns at step t are a
prefix; step t runs at width U[t] = ceil(K_{t-1}/8) and dead columns inside
the width are held exactly by injecting +BIG into the z-gate pre-activation
from a packed mask row (sigmoid(BIG) == 1.0 in fp32, so h' = h).

Layout: feature-major ([feature, batch]); no transposes on device. Biases
ride the matmuls via two ones-rows in the moving operand (hi/lo bf16 split).
All t-loop matmuls are bf16 at ragged width. The two GRU layers run skewed
(layer0@t with layer1@t-1) so every elementwise instruction covers both
layers at once, halving per-op overheads.
"""

import numpy as np
import ml_dtypes

import concourse.bacc as bacc
import concourse.mybir as mybir
from concourse.tile import TileContext
from concourse import bass_utils
from bass_rust import add_dep_helper

BF16 = ml_dtypes.bfloat16
F32 = np.float32

D_IN = 364
EMB1 = 128
H = 96
N_ACT = 8
NCORES = 8
SLOPE = 0.01
BIG = 30.0
DPAD = 384  # D_IN padded to 3*128; row D_IN carries ones for the b1 bias
PH1_COLS = 1024  # phase-1 outer DMA tile (columns)
PH1_MM = 512  # phase-1 matmul chunk (columns)
ALU = mybir.AluOpType
ACTF = mybir.ActivationFunctionType


def _gate(w, g):
    # rows of a [3H, *] torch-style stacked gate tensor; g in {0:r, 1:z, 2:n}
    return w[g * H:(g + 1) * H]


def _hilo(b):
    hi = b.astype(BF16).astype(F32)
    return hi, b - hi


def prep_host(x, lengths, h0, emb_w1, emb_b1, emb_w2, emb_b2,
              w_ih0, w_hh0, b_ih0, b_hh0, w_ih1, w_hh1, b_ih1, b_hh1,
              fc_w, fc_b):
    """Pack inputs host-side. Returns (meta, in_maps)."""
    x = np.asarray(x, F32)
    lengths = np.asarray(lengths).astype(np.int64)
    h0 = np.asarray(h0, F32)
    T, B, _ = x.shape
    NB = B // NCORES
    Tmax = int(lengths.max())

    # per-step widths: K_t live columns globally; step t is packed/processed
    # at width U[t] = W[t-1] (W non-increasing) so that merged ops for
    # (l0@t, l1@t-1) share one width
    K = (lengths[None, :] > np.arange(Tmax)[:, None]).sum(1)  # [Tmax]
    W = ((K + NCORES - 1) // NCORES).astype(np.int64)
    U = np.concatenate([W[:1], W[:-1]])  # U[t] = W[t-1], U[0] = W[0]
    OFF = np.zeros(Tmax + 1, np.int64)
    OFF[1:] = np.cumsum(U)
    NPACK = int(OFF[-1])

    # ---- shared (replicated) weight arrays ----
    emb_w1 = np.asarray(emb_w1, F32)
    emb_b1 = np.asarray(emb_b1, F32)
    emb_w2 = np.asarray(emb_w2, F32)
    emb_b2 = np.asarray(emb_b2, F32)
    fc_w = np.asarray(fc_w, F32)
    fc_b = np.asarray(fc_b, F32)
    w_ih = [np.asarray(w_ih0, F32), np.asarray(w_ih1, F32)]
    w_hh = [np.asarray(w_hh0, F32), np.asarray(w_hh1, F32)]
    b_ih = [np.asarray(b_ih0, F32), np.asarray(b_ih1, F32)]
    b_hh = [np.asarray(b_hh0, F32), np.asarray(b_hh1, F32)]

    # embedding layer 1, transposed + padded; row D_IN of the input carries
    # ones so the padded weight row holds b1
    ew1t = np.zeros((DPAD, EMB1), F32)
    ew1t[:D_IN] = emb_w1.T
    ew1t[D_IN] = emb_b1
    ew1t = ew1t.astype(BF16)
    ew2t = emb_w2.T.astype(BF16)  # [128, 96]

    # b2 folds into layer-0 x-side biases; r,z h-side biases fold into the
    # x side too (n keeps b_hh on the h side: r multiplies it separately)
    bx = [b_ih[0] + w_ih[0] @ emb_b2, b_ih[1].copy()]
    for l in range(2):
        bx[l][0:2 * H] += b_hh[l][0:2 * H]

    # gate lhsT blocks [98, 96]: rows 0:96 = W.T, rows 96/97 = bias hi/lo
    def gate_block(wmat, bias, g):
        blk = np.zeros((H + 2, H), F32)
        blk[:H] = _gate(wmat, g).T
        blk[H], blk[H + 1] = _hilo(_gate(bias, g))
        return blk

    wx = np.zeros((2, 3, H + 2, H), F32)
    wh = np.zeros((2, 3, H + 2, H), F32)
    zero_b = np.zeros(3 * H, F32)
    for l in range(2):
        for g in range(3):
            wx[l, g] = gate_block(w_ih[l], bx[l], g)
            wh[l, g] = gate_block(w_hh[l], b_hh[l] if g == 2 else zero_b, g)
    wx = wx.astype(BF16)
    wh = wh.astype(BF16)

    mz = np.full((1, H), BIG, F32).astype(BF16)  # dead-column z injection

    fct = np.zeros((H + 1, N_ACT), F32)  # fc head stays fp32 (one matmul)
    fct[:H] = fc_w.T
    fct[H] = fc_b

    shared = dict(ew1t=ew1t, ew2t=ew2t, wx=wx, wh=wh, mz=mz, fct=fct)

    # ---- per-core packed inputs ----
    in_maps = []
    for c in range(NCORES):
        xc = x[:, c::NCORES, :]  # [T, NB, D] view
        lens_c = lengths[c::NCORES]  # sorted desc
        cols = [xc[t, :U[t], :] for t in range(Tmax)]
        xcat = np.concatenate(cols, axis=0) if cols else np.zeros((0, D_IN), F32)
        xp = np.zeros((DPAD, NPACK), BF16)
        xp[:D_IN] = np.ascontiguousarray(xcat.T).astype(BF16)
        xp[D_IN] = BF16(1.0)
        mp = np.zeros((3, NPACK), BF16)
        mp[0] = BF16(1.0)
        mp[1] = BF16(1.0)
        dead = (np.concatenate([(lens_c[:U[t]] <= t) for t in range(Tmax)])
                if Tmax else np.zeros(0, bool))
        mp[2] = dead.astype(BF16)
        h0t = np.ascontiguousarray(
            h0[:, c::NCORES, :].transpose(0, 2, 1)).astype(BF16)  # [2, 96, NB]
        ones_nb = np.ones((2, NB), BF16)
        in_maps.append(dict(xp=xp, mp=mp, h0t=h0t, ones_nb=ones_nb, **shared))

    meta = dict(T=T, B=B, NB=NB, Tmax=Tmax, NPACK=NPACK,
                U=U.tolist(), OFF=OFF.tolist())
    return meta, in_maps


def build_program(meta):
    NB = meta["NB"]
    Tmax = meta["Tmax"]
    NPACK = meta["NPACK"]
    U = meta["U"]
    OFF = meta["OFF"]
    f32 = mybir.dt.float32
    bf16 = mybir.dt.bfloat16

    nc = bacc.Bacc(trn_type="TRN2", target_bir_lowering=False, debug=False,
                   num_devices=NCORES)

    # DRAM I/O
    d_xp = nc.dram_tensor("xp", [DPAD, NPACK], bf16, kind="ExternalInput").ap()
    d_mp = nc.dram_tensor("mp", [3, NPACK], bf16, kind="ExternalInput").ap()
    d_h0t = nc.dram_tensor("h0t", [2, H, NB], bf16, kind="ExternalInput").ap()
    d_ones = nc.dram_tensor("ones_nb", [2, NB], bf16, kind="ExternalInput").ap()
    d_ew1t = nc.dram_tensor("ew1t", [DPAD, EMB1], bf16, kind="ExternalInput").ap()
    d_ew2t = nc.dram_tensor("ew2t", [EMB1, H], bf16, kind="ExternalInput").ap()
    d_wx = nc.dram_tensor("wx", [2, 3, H + 2, H], bf16, kind="ExternalInput").ap()
    d_wh = nc.dram_tensor("wh", [2, 3, H + 2, H], bf16, kind="ExternalInput").ap()
    d_mz = nc.dram_tensor("mz", [1, H], bf16, kind="ExternalInput").ap()
    d_fct = nc.dram_tensor("fct", [H + 1, N_ACT], f32, kind="ExternalInput").ap()
    d_outT = nc.dram_tensor("outT", [N_ACT, NB], f32, kind="ExternalOutput").ap()
    d_hlast = nc.dram_tensor("hlast", [2, H, NB], f32, kind="ExternalOutput").ap()

    with TileContext(nc) as tc:
        with (
            tc.tile_pool(name="persist", bufs=1) as persist,
            tc.tile_pool(name="xin", bufs=3) as xin,
            tc.tile_pool(name="a1p", bufs=3) as a1p,
            tc.tile_pool(name="gtmp", bufs=4) as gtmp,
            tc.tile_pool(name="psum", bufs=2, space="PSUM") as psp,
        ):
            # ---- persistent tiles ----
            E = persist.tile([H + 2, NPACK], bf16)  # packed e + 2 ones rows
            M1 = persist.tile([1, NPACK], bf16)  # dead-column mask row
            nc.sync.dma_start(out=E[H:H + 2, :], in_=d_mp[0:2, :])
            nc.sync.dma_start(out=M1, in_=d_mp[2:3, :])
            EW1 = persist.tile([128, 3, EMB1], bf16)
            for ci in range(3):
                nc.sync.dma_start(out=EW1[:, ci, :],
                                  in_=d_ew1t[ci * 128:(ci + 1) * 128, :])
            EW2 = persist.tile([EMB1, H], bf16)
            nc.sync.dma_start(out=EW2, in_=d_ew2t)
            WX = persist.tile([H + 2, 2, 3, H], bf16)
            WHT = persist.tile([H + 2, 2, 3, H], bf16)
            for l in range(2):
                for g in range(3):
                    nc.sync.dma_start(out=WX[:, l, g, :], in_=d_wx[l, g])
                    nc.sync.dma_start(out=WHT[:, l, g, :], in_=d_wh[l, g])
            MZ = persist.tile([1, H], bf16)
            nc.sync.dma_start(out=MZ, in_=d_mz)
            FCT = persist.tile([H + 1, N_ACT], f32)
            nc.sync.dma_start(out=FCT, in_=d_fct)
            # HH: rows 0:96 = h (dim1: layer), rows 96/97 = ones
            HH = persist.tile([H + 2, 2, NB], bf16)
            for l in range(2):
                nc.sync.dma_start(out=HH[0:H, l, :], in_=d_h0t[l])
                nc.sync.dma_start(out=HH[H:H + 2, l, :], in_=d_ones)

            # ---- phase 1: embedding MLP over all packed columns ----
            for c0 in range(0, NPACK, PH1_COLS):
                c1 = min(NPACK, c0 + PH1_COLS)
                w = c1 - c0
                xt = xin.tile([128, 3, PH1_COLS], bf16, tag="xt")
                for ci in range(3):
                    nc.sync.dma_start(out=xt[:, ci, 0:w],
                                      in_=d_xp[ci * 128:(ci + 1) * 128, c0:c1])
                for j0 in range(0, w, PH1_MM):
                    j1 = min(w, j0 + PH1_MM)
                    jw = j1 - j0
                    pa = psp.tile([128, PH1_MM], f32, tag="psA")
                    for ci in range(3):
                        nc.tensor.matmul(pa[:, 0:jw], EW1[:, ci, :],
                                         xt[:, ci, j0:j1],
                                         start=(ci == 0), stop=(ci == 2))
                    a0 = a1p.tile([128, PH1_MM], f32, tag="a0")
                    nc.scalar.copy(a0[:, 0:jw], pa[:, 0:jw])
                    a1 = a1p.tile([128, PH1_MM], bf16, tag="a1")
                    # LeakyReLU: max(v, 0.01*v); b1 already in psum (ones row)
                    nc.vector.scalar_tensor_tensor(
                        out=a1[:, 0:jw], in0=a0[:, 0:jw], scalar=SLOPE,
                        in1=a0[:, 0:jw], op0=ALU.mult, op1=ALU.max)
                    pb = psp.tile([H, PH1_MM], f32, tag="psB")
                    nc.tensor.matmul(pb[:, 0:jw], EW2, a1[:, 0:jw],
                                     start=True, stop=True)
                    nc.scalar.copy(E[0:H, c0 + j0:c0 + j1], pb[:, 0:jw])

            # ---- phase 2: skewed fused GRU steps ----
            # merged step s computes l0@s (if s<Tmax) + l1@s-1 (if s>0),
            # both at width U[s] (U[s] <= U[s-1], so l1 ops are in-range).
            # PSUM layouts keep every psum-reading instruction in one bank:
            #   psRZ: bank A = [r0, z0], bank B = [r1, z1]
            #   psN:  bank A = [nh0, nh1], bank B = [nx0, nx1]
            # SBUF RZ layout: [r0, r1, z0, z1] so r/z pairs are contiguous.
            for s in range(Tmax + 1):
                u = U[s] if s < Tmax else U[Tmax - 1]
                off = OFF[s] if s < Tmax else None
                off1 = OFF[s - 1] if s > 0 else None
                do0 = s < Tmax
                do1 = s > 0
                la = 0 if do0 else 1
                lb = 2 if do1 else 1

                small = u <= 128
                # For u<=128 all four r/z regions fit one 2KB psum bank: one
                # accumulation group and ONE merged sigmoid. Otherwise two
                # banks / two sigmoids. psRZ layout: [r0, z0, r1, z1];
                # psN: [nh0, nh1, nx0, nx1]; RZ (sbuf) mirrors psRZ order.
                uu = 128 if small else NB
                psRZ = psp.tile([H, 4, uu], f32, tag="psA")
                psN = psp.tile([H, 4, uu], f32, tag="psB")
                rz0, rz1, bnh, bnx = [], [], [], []
                if do0:
                    xr = E[0:H + 2, off:off + u]
                    hr = HH[0:H + 2, 0, 0:u]
                    rz0 += [
                        (psRZ[:, 0, 0:u], WX[:, 0, 0, :], xr),
                        (psRZ[:, 1, 0:u], WX[:, 0, 1, :], xr),
                        (psRZ[:, 1, 0:u], MZ, M1[0:1, off:off + u]),
                    ]
                    bnx.append((psN[:, 2, 0:u], WX[:, 0, 2, :], xr))
                if do1:
                    xr1 = HH[0:H + 2, 0, 0:u]  # h0 before this step's update
                    rz1 += [
                        (psRZ[:, 2, 0:u], WX[:, 1, 0, :], xr1),
                        (psRZ[:, 3, 0:u], WX[:, 1, 1, :], xr1),
                        (psRZ[:, 3, 0:u], MZ, M1[0:1, off1:off1 + u]),
                    ]
                    bnx.append((psN[:, 3, 0:u], WX[:, 1, 2, :], xr1))
                # h-side (on the chain): r/z first so sigmoids unblock, n last
                if do0:
                    hr = HH[0:H + 2, 0, 0:u]
                    rz0 += [(psRZ[:, 0, 0:u], WHT[:, 0, 0, :], hr),
                            (psRZ[:, 1, 0:u], WHT[:, 0, 1, :], hr)]
                    bnh.append((psN[:, 0, 0:u], WHT[:, 0, 2, :], hr))
                if do1:
                    hr1 = HH[0:H + 2, 1, 0:u]
                    rz1 += [(psRZ[:, 2, 0:u], WHT[:, 1, 0, :], hr1),
                            (psRZ[:, 3, 0:u], WHT[:, 1, 1, :], hr1)]
                    bnh.append((psN[:, 1, 0:u], WHT[:, 1, 2, :], hr1))
                if small:
                    banks = [rz0 + rz1, bnx + bnh]
                else:
                    banks = [rz0, rz1, bnx, bnh]
                for bank in banks:
                    prev = None
                    for i, (o, lhs, rhs) in enumerate(bank):
                        inst = nc.tensor.matmul(o, lhs, rhs, start=(i == 0),
                                                stop=(i == len(bank) - 1),
                                                skip_group_check=True)
                        if prev is not None:
                            add_dep_helper(inst.ins, prev.ins, False,
                                           "psum bank group order")
                        prev = inst

                RZ = gtmp.tile([H, 4, NB], bf16, tag="RZ")  # [r0,z0,r1,z1]
                if small and do0 and do1:
                    nc.scalar.activation(RZ[:, 0:4, 0:u], psRZ[:, 0:4, 0:u],
                                         ACTF.Sigmoid)
                else:
                    for li in range(la, lb):
                        nc.scalar.activation(RZ[:, 2 * li:2 * li + 2, 0:u],
                                             psRZ[:, 2 * li:2 * li + 2, 0:u],
                                             ACTF.Sigmoid)
                T2 = gtmp.tile([H, 2, NB], f32, tag="T2")
                nc.vector.tensor_tensor(
                    out=T2[:, la:lb, 0:u], in0=psN[:, la:lb, 0:u],
                    in1=RZ[:, 2 * la:2 * lb:2, 0:u], op=ALU.mult)
                PRE = gtmp.tile([H, 2, NB], f32, tag="PRE")
                nc.vector.tensor_tensor(
                    out=PRE[:, la:lb, 0:u], in0=T2[:, la:lb, 0:u],
                    in1=psN[:, 2 + la:2 + lb, 0:u], op=ALU.add)
                NT = gtmp.tile([H, 2, NB], bf16, tag="NT")
                nc.scalar.activation(NT[:, la:lb, 0:u], PRE[:, la:lb, 0:u],
                                     ACTF.Tanh)
                WT = gtmp.tile([H, 2, NB], bf16, tag="WT")
                nc.vector.tensor_tensor(
                    out=WT[:, la:lb, 0:u], in0=HH[0:H, la:lb, 0:u],
                    in1=NT[:, la:lb, 0:u], op=ALU.subtract)
                VT = gtmp.tile([H, 2, NB], bf16, tag="VT")
                nc.vector.tensor_tensor(
                    out=VT[:, la:lb, 0:u], in0=RZ[:, 2 * la + 1:2 * lb:2, 0:u],
                    in1=WT[:, la:lb, 0:u], op=ALU.mult)
                nc.vector.tensor_tensor(
                    out=HH[0:H, la:lb, 0:u], in0=NT[:, la:lb, 0:u],
                    in1=VT[:, la:lb, 0:u], op=ALU.add)

            # ---- fc head + outputs ----
            HL = persist.tile([H, 2, NB], f32)
            nc.scalar.copy(HL[:, 0, :], HH[0:H, 0, :])
            nc.scalar.copy(HL[:, 1, :], HH[0:H, 1, :])
            nc.sync.dma_start(out=d_hlast[0], in_=HL[:, 0, :])
            nc.sync.dma_start(out=d_hlast[1], in_=HL[:, 1, :])
            LH = persist.tile([H + 1, NB], f32)
            nc.vector.memset(LH[H:H + 1, :], 1.0)
            nc.vector.scalar_tensor_tensor(
                out=LH[0:H, :], in0=HL[:, 1, :], scalar=SLOPE,
                in1=HL[:, 1, :], op0=ALU.mult, op1=ALU.max)
            psO = psp.tile([N_ACT, NB], f32, tag="psB")
            nc.tensor.matmul(psO, FCT, LH, start=True, stop=True)
            OS = persist.tile([N_ACT, NB], f32)
            nc.vector.tensor_copy(OS, psO)
            nc.sync.dma_start(out=d_outT, in_=OS)

    nc.compile()
    return nc


def kernel(**inputs):
    meta, in_maps = prep_host(**inputs)
    nc = build_program(meta)
    res = bass_utils.run_bass_kernel_spmd(nc, in_maps,
                                          core_ids=list(range(NCORES)))
    B = meta["B"]
    out = np.zeros((B, N_ACT), F32)
    h = np.zeros((2, B, H), F32)
    for c in range(NCORES):
        r = res.results[c]
        out[c::NCORES, :] = r["outT"].T
        h[0, c::NCORES, :] = r["hlast"][0].T
        h[1, c::NCORES, :] = r["hlast"][1].T
    return out, h


# revision 29
# speedup vs baseline: 1.0781x; 1.0014x over previous
"""Trainium2 Bass kernel for a 2-layer GRU model over ragged (packed) sequences.

Model (PyTorch gate order r,z,n):
  e   = LeakyReLU(x @ W1.T + b1, 0.01) @ W2.T + b2          # embedding MLP
  hs1 = GRU0(e),  hs2 = GRU1(hs1)      (masked: state held past seq end)
  out = LeakyReLU(h2_last, 0.01) @ fc_w.T + fc_b
  returns (out[B,8], h[2,B,96])

Distribution: data-parallel over batch, strided (core c gets b = c+8j).
`lengths` is sorted descending, so each core's live columns at step t are a
prefix; step t runs at width U[t] = ceil(K_{t-1}/8) and dead columns inside
the width are held exactly by injecting +BIG into the z-gate pre-activation
from a packed mask row (sigmoid(BIG) == 1.0 in fp32, so h' = h).

Layout: feature-major ([feature, batch]); no transposes on device. Biases
ride the matmuls via two ones-rows in the moving operand (hi/lo bf16 split).
All t-loop matmuls are bf16 at ragged width. The two GRU layers run skewed
(layer0@t with layer1@t-1) so every elementwise instruction covers both
layers at once, halving per-op overheads.
"""

import numpy as np
import ml_dtypes

import concourse.bacc as bacc
import concourse.mybir as mybir
from concourse.tile import TileContext
from concourse import bass_utils
from bass_rust import add_dep_helper

BF16 = ml_dtypes.bfloat16
F32 = np.float32

D_IN = 364
EMB1 = 128
H = 96
N_ACT = 8
NCORES = 8
SLOPE = 0.01
BIG = 30.0
DPAD = 384  # D_IN padded to 3*128; row D_IN carries ones for the b1 bias
PH1_COLS = 1024  # phase-1 outer DMA tile (columns)
PH1_MM = 512  # phase-1 matmul chunk (columns)
ALU = mybir.AluOpType
ACTF = mybir.ActivationFunctionType


def _gate(w, g):
    # rows of a [3H, *] torch-style stacked gate tensor; g in {0:r, 1:z, 2:n}
    return w[g * H:(g + 1) * H]


def _hilo(b):
    hi = b.astype(BF16).astype(F32)
    return hi, b - hi


def prep_host(x, lengths, h0, emb_w1, emb_b1, emb_w2, emb_b2,
              w_ih0, w_hh0, b_ih0, b_hh0, w_ih1, w_hh1, b_ih1, b_hh1,
              fc_w, fc_b):
    """Pack inputs host-side. Returns (meta, in_maps)."""
    x = np.asarray(x, F32)
    lengths = np.asarray(lengths).astype(np.int64)
    h0 = np.asarray(h0, F32)
    T, B, _ = x.shape
    NB = B // NCORES
    Tmax = int(lengths.max())

    # per-step widths: K_t live columns globally; step t is packed/processed
    # at width U[t] = W[t-1] (W non-increasing) so that merged ops for
    # (l0@t, l1@t-1) share one width
    K = (lengths[None, :] > np.arange(Tmax)[:, None]).sum(1)  # [Tmax]
    W = ((K + NCORES - 1) // NCORES).astype(np.int64)
    U = np.concatenate([W[:1], W[:-1]])  # U[t] = W[t-1], U[0] = W[0]
    OFF = np.zeros(Tmax + 1, np.int64)
    OFF[1:] = np.cumsum(U)
    NPACK = int(OFF[-1])

    # ---- shared (replicated) weight arrays ----
    emb_w1 = np.asarray(emb_w1, F32)
    emb_b1 = np.asarray(emb_b1, F32)
    emb_w2 = np.asarray(emb_w2, F32)
    emb_b2 = np.asarray(emb_b2, F32)
    fc_w = np.asarray(fc_w, F32)
    fc_b = np.asarray(fc_b, F32)
    w_ih = [np.asarray(w_ih0, F32), np.asarray(w_ih1, F32)]
    w_hh = [np.asarray(w_hh0, F32), np.asarray(w_hh1, F32)]
    b_ih = [np.asarray(b_ih0, F32), np.asarray(b_ih1, F32)]
    b_hh = [np.asarray(b_hh0, F32), np.asarray(b_hh1, F32)]

    # embedding layer 1, transposed + padded; row D_IN of the input carries
    # ones so the padded weight row holds b1
    ew1t = np.zeros((DPAD, EMB1), F32)
    ew1t[:D_IN] = emb_w1.T
    ew1t[D_IN] = emb_b1
    ew1t = ew1t.astype(BF16)
    ew2t = emb_w2.T.astype(BF16)  # [128, 96]

    # b2 folds into layer-0 x-side biases; r,z h-side biases fold into the
    # x side too (n keeps b_hh on the h side: r multiplies it separately)
    bx = [b_ih[0] + w_ih[0] @ emb_b2, b_ih[1].copy()]
    for l in range(2):
        bx[l][0:2 * H] += b_hh[l][0:2 * H]

    # gate lhsT blocks [98, 96]: rows 0:96 = W.T, rows 96/97 = bias hi/lo
    def gate_block(wmat, bias, g):
        blk = np.zeros((H + 2, H), F32)
        blk[:H] = _gate(wmat, g).T
        blk[H], blk[H + 1] = _hilo(_gate(bias, g))
        return blk

    wx = np.zeros((2, 3, H + 2, H), F32)
    wh = np.zeros((2, 3, H + 2, H), F32)
    zero_b = np.zeros(3 * H, F32)
    for l in range(2):
        for g in range(3):
            wx[l, g] = gate_block(w_ih[l], bx[l], g)
            wh[l, g] = gate_block(w_hh[l], b_hh[l] if g == 2 else zero_b, g)
    wx = wx.astype(BF16)
    wh = wh.astype(BF16)

    mz = np.full((1, H), BIG, F32).astype(BF16)  # dead-column z injection

    fct = np.zeros((H + 1, N_ACT), F32)  # fc head stays fp32 (one matmul)
    fct[:H] = fc_w.T
    fct[H] = fc_b

    shared = dict(ew1t=ew1t, ew2t=ew2t, wx=wx, wh=wh, mz=mz, fct=fct)

    # ---- per-core packed inputs ----
    in_maps = []
    for c in range(NCORES):
        xc = x[:, c::NCORES, :]  # [T, NB, D] view
        lens_c = lengths[c::NCORES]  # sorted desc
        cols = [xc[t, :U[t], :] for t in range(Tmax)]
        xcat = np.concatenate(cols, axis=0) if cols else np.zeros((0, D_IN), F32)
        xp = np.zeros((DPAD, NPACK), BF16)
        xp[:D_IN] = np.ascontiguousarray(xcat.T).astype(BF16)
        xp[D_IN] = BF16(1.0)
        mp = np.zeros((3, NPACK), BF16)
        mp[0] = BF16(1.0)
        mp[1] = BF16(1.0)
        dead = (np.concatenate([(lens_c[:U[t]] <= t) for t in range(Tmax)])
                if Tmax else np.zeros(0, bool))
        mp[2] = dead.astype(BF16)
        h0t = np.ascontiguousarray(
            h0[:, c::NCORES, :].transpose(0, 2, 1)).astype(BF16)  # [2, 96, NB]
        ones_nb = np.ones((2, NB), BF16)
        in_maps.append(dict(xp=xp, mp=mp, h0t=h0t, ones_nb=ones_nb, **shared))

    meta = dict(T=T, B=B, NB=NB, Tmax=Tmax, NPACK=NPACK,
                U=U.tolist(), OFF=OFF.tolist())
    return meta, in_maps


def build_program(meta):
    NB = meta["NB"]
    Tmax = meta["Tmax"]
    NPACK = meta["NPACK"]
    U = meta["U"]
    OFF = meta["OFF"]
    f32 = mybir.dt.float32
    bf16 = mybir.dt.bfloat16

    nc = bacc.Bacc(trn_type="TRN2", target_bir_lowering=False, debug=False,
                   num_devices=NCORES)

    # DRAM I/O
    d_xp = nc.dram_tensor("xp", [DPAD, NPACK], bf16, kind="ExternalInput").ap()
    d_mp = nc.dram_tensor("mp", [3, NPACK], bf16, kind="ExternalInput").ap()
    d_h0t = nc.dram_tensor("h0t", [2, H, NB], bf16, kind="ExternalInput").ap()
    d_ones = nc.dram_tensor("ones_nb", [2, NB], bf16, kind="ExternalInput").ap()
    d_ew1t = nc.dram_tensor("ew1t", [DPAD, EMB1], bf16, kind="ExternalInput").ap()
    d_ew2t = nc.dram_tensor("ew2t", [EMB1, H], bf16, kind="ExternalInput").ap()
    d_wx = nc.dram_tensor("wx", [2, 3, H + 2, H], bf16, kind="ExternalInput").ap()
    d_wh = nc.dram_tensor("wh", [2, 3, H + 2, H], bf16, kind="ExternalInput").ap()
    d_mz = nc.dram_tensor("mz", [1, H], bf16, kind="ExternalInput").ap()
    d_fct = nc.dram_tensor("fct", [H + 1, N_ACT], f32, kind="ExternalInput").ap()
    d_outT = nc.dram_tensor("outT", [N_ACT, NB], f32, kind="ExternalOutput").ap()
    d_hlast = nc.dram_tensor("hlast", [2, H, NB], f32, kind="ExternalOutput").ap()

    with TileContext(nc) as tc:
        with (
            tc.tile_pool(name="persist", bufs=1) as persist,
            tc.tile_pool(name="xin", bufs=3) as xin,
            tc.tile_pool(name="a1p", bufs=3) as a1p,
            tc.tile_pool(name="gtmp", bufs=5) as gtmp,
            tc.tile_pool(name="psum", bufs=2, space="PSUM") as psp,
        ):
            # ---- persistent tiles ----
            E = persist.tile([H + 2, NPACK], bf16)  # packed e + 2 ones rows
            M1 = persist.tile([1, NPACK], bf16)  # dead-column mask row
            nc.sync.dma_start(out=E[H:H + 2, :], in_=d_mp[0:2, :])
            nc.sync.dma_start(out=M1, in_=d_mp[2:3, :])
            EW1 = persist.tile([128, 3, EMB1], bf16)
            for ci in range(3):
                nc.sync.dma_start(out=EW1[:, ci, :],
                                  in_=d_ew1t[ci * 128:(ci + 1) * 128, :])
            EW2 = persist.tile([EMB1, H], bf16)
            nc.sync.dma_start(out=EW2, in_=d_ew2t)
            WX = persist.tile([H + 2, 2, 3, H], bf16)
            WHT = persist.tile([H + 2, 2, 3, H], bf16)
            for l in range(2):
                for g in range(3):
                    nc.sync.dma_start(out=WX[:, l, g, :], in_=d_wx[l, g])
                    nc.sync.dma_start(out=WHT[:, l, g, :], in_=d_wh[l, g])
            MZ = persist.tile([1, H], bf16)
            nc.sync.dma_start(out=MZ, in_=d_mz)
            FCT = persist.tile([H + 1, N_ACT], f32)
            nc.sync.dma_start(out=FCT, in_=d_fct)
            # HH: rows 0:96 = h (dim1: layer), rows 96/97 = ones
            HH = persist.tile([H + 2, 2, NB], bf16)
            for l in range(2):
                nc.sync.dma_start(out=HH[0:H, l, :], in_=d_h0t[l])
                nc.sync.dma_start(out=HH[H:H + 2, l, :], in_=d_ones)

            # ---- phase 1: embedding MLP over all packed columns ----
            for c0 in range(0, NPACK, PH1_COLS):
                c1 = min(NPACK, c0 + PH1_COLS)
                w = c1 - c0
                xt = xin.tile([128, 3, PH1_COLS], bf16, tag="xt")
                for ci in range(3):
                    nc.sync.dma_start(out=xt[:, ci, 0:w],
                                      in_=d_xp[ci * 128:(ci + 1) * 128, c0:c1])
                for j0 in range(0, w, PH1_MM):
                    j1 = min(w, j0 + PH1_MM)
                    jw = j1 - j0
                    pa = psp.tile([128, PH1_MM], f32, tag="psA")
                    for ci in range(3):
                        nc.tensor.matmul(pa[:, 0:jw], EW1[:, ci, :],
                                         xt[:, ci, j0:j1],
                                         start=(ci == 0), stop=(ci == 2))
                    a0 = a1p.tile([128, PH1_MM], f32, tag="a0")
                    nc.scalar.copy(a0[:, 0:jw], pa[:, 0:jw])
                    a1 = a1p.tile([128, PH1_MM], bf16, tag="a1")
                    # LeakyReLU: max(v, 0.01*v); b1 already in psum (ones row)
                    nc.vector.scalar_tensor_tensor(
                        out=a1[:, 0:jw], in0=a0[:, 0:jw], scalar=SLOPE,
                        in1=a0[:, 0:jw], op0=ALU.mult, op1=ALU.max)
                    pb = psp.tile([H, PH1_MM], f32, tag="psB")
                    nc.tensor.matmul(pb[:, 0:jw], EW2, a1[:, 0:jw],
                                     start=True, stop=True)
                    nc.scalar.copy(E[0:H, c0 + j0:c0 + j1], pb[:, 0:jw])

            # ---- phase 2: skewed fused GRU steps ----
            # merged step s computes l0@s (if s<Tmax) + l1@s-1 (if s>0),
            # both at width U[s] (U[s] <= U[s-1], so l1 ops are in-range).
            # PSUM layouts keep every psum-reading instruction in one bank:
            #   psRZ: bank A = [r0, z0], bank B = [r1, z1]
            #   psN:  bank A = [nh0, nh1], bank B = [nx0, nx1]
            # SBUF RZ layout: [r0, r1, z0, z1] so r/z pairs are contiguous.
            for s in range(Tmax + 1):
                u = U[s] if s < Tmax else U[Tmax - 1]
                off = OFF[s] if s < Tmax else None
                off1 = OFF[s - 1] if s > 0 else None
                do0 = s < Tmax
                do1 = s > 0
                la = 0 if do0 else 1
                lb = 2 if do1 else 1

                small = u <= 128
                # For u<=128 all four r/z regions fit one 2KB psum bank: one
                # accumulation group and ONE merged sigmoid. Otherwise two
                # banks / two sigmoids. psRZ layout: [r0, z0, r1, z1];
                # psN: [nh0, nh1, nx0, nx1]; RZ (sbuf) mirrors psRZ order.
                uu = 128 if small else NB
                psRZ = psp.tile([H, 4, uu], f32, tag="psA")
                psN = psp.tile([H, 4, uu], f32, tag="psB")
                rz0, rz1, bnh, bnx = [], [], [], []
                if do0:
                    xr = E[0:H + 2, off:off + u]
                    hr = HH[0:H + 2, 0, 0:u]
                    rz0 += [
                        (psRZ[:, 0, 0:u], WX[:, 0, 0, :], xr),
                        (psRZ[:, 1, 0:u], WX[:, 0, 1, :], xr),
                        (psRZ[:, 1, 0:u], MZ, M1[0:1, off:off + u]),
                    ]
                    bnx.append((psN[:, 2, 0:u], WX[:, 0, 2, :], xr))
                if do1:
                    xr1 = HH[0:H + 2, 0, 0:u]  # h0 before this step's update
                    rz1 += [
                        (psRZ[:, 2, 0:u], WX[:, 1, 0, :], xr1),
                        (psRZ[:, 3, 0:u], WX[:, 1, 1, :], xr1),
                        (psRZ[:, 3, 0:u], MZ, M1[0:1, off1:off1 + u]),
                    ]
                    bnx.append((psN[:, 3, 0:u], WX[:, 1, 2, :], xr1))
                # h-side (on the chain): r/z first so sigmoids unblock, n last
                if do0:
                    hr = HH[0:H + 2, 0, 0:u]
                    rz0 += [(psRZ[:, 0, 0:u], WHT[:, 0, 0, :], hr),
                            (psRZ[:, 1, 0:u], WHT[:, 0, 1, :], hr)]
                    bnh.append((psN[:, 0, 0:u], WHT[:, 0, 2, :], hr))
                if do1:
                    hr1 = HH[0:H + 2, 1, 0:u]
                    rz1 += [(psRZ[:, 2, 0:u], WHT[:, 1, 0, :], hr1),
                            (psRZ[:, 3, 0:u], WHT[:, 1, 1, :], hr1)]
                    bnh.append((psN[:, 1, 0:u], WHT[:, 1, 2, :], hr1))
                if small:
                    banks = [rz0 + rz1, bnx + bnh]
                else:
                    banks = [rz0, rz1, bnx, bnh]
                for bank in banks:
                    prev = None
                    for i, (o, lhs, rhs) in enumerate(bank):
                        inst = nc.tensor.matmul(o, lhs, rhs, start=(i == 0),
                                                stop=(i == len(bank) - 1),
                                                skip_group_check=True)
                        if prev is not None:
                            add_dep_helper(inst.ins, prev.ins, False,
                                           "psum bank group order")
                        prev = inst

                RZ = gtmp.tile([H, 4, NB], bf16, tag="RZ")  # [r0,z0,r1,z1]
                if small and do0 and do1:
                    nc.scalar.activation(RZ[:, 0:4, 0:u], psRZ[:, 0:4, 0:u],
                                         ACTF.Sigmoid)
                else:
                    for li in range(la, lb):
                        nc.scalar.activation(RZ[:, 2 * li:2 * li + 2, 0:u],
                                             psRZ[:, 2 * li:2 * li + 2, 0:u],
                                             ACTF.Sigmoid)
                T2 = gtmp.tile([H, 2, NB], f32, tag="T2")
                nc.vector.tensor_tensor(
                    out=T2[:, la:lb, 0:u], in0=psN[:, la:lb, 0:u],
                    in1=RZ[:, 2 * la:2 * lb:2, 0:u], op=ALU.mult)
                PRE = gtmp.tile([H, 2, NB], f32, tag="PRE")
                nc.vector.tensor_tensor(
                    out=PRE[:, la:lb, 0:u], in0=T2[:, la:lb, 0:u],
                    in1=psN[:, 2 + la:2 + lb, 0:u], op=ALU.add)
                NT = gtmp.tile([H, 2, NB], bf16, tag="NT")
                nc.scalar.activation(NT[:, la:lb, 0:u], PRE[:, la:lb, 0:u],
                                     ACTF.Tanh)
                WT = gtmp.tile([H, 2, NB], bf16, tag="WT")
                nc.vector.tensor_tensor(
                    out=WT[:, la:lb, 0:u], in0=HH[0:H, la:lb, 0:u],
                    in1=NT[:, la:lb, 0:u], op=ALU.subtract)
                VT = gtmp.tile([H, 2, NB], bf16, tag="VT")
                nc.vector.tensor_tensor(
                    out=VT[:, la:lb, 0:u], in0=RZ[:, 2 * la + 1:2 * lb:2, 0:u],
                    in1=WT[:, la:lb, 0:u], op=ALU.mult)
                nc.vector.tensor_tensor(
                    out=HH[0:H, la:lb, 0:u], in0=NT[:, la:lb, 0:u],
                    in1=VT[:, la:lb, 0:u], op=ALU.add)

            # ---- fc head + outputs ----
            HL = persist.tile([H, 2, NB], f32)
            nc.scalar.copy(HL[:, 0, :], HH[0:H, 0, :])
            nc.scalar.copy(HL[:, 1, :], HH[0:H, 1, :])
            nc.sync.dma_start(out=d_hlast[0], in_=HL[:, 0, :])
            nc.sync.dma_start(out=d_hlast[1], in_=HL[:, 1, :])
            LH = persist.tile([H + 1, NB], f32)
            nc.vector.memset(LH[H:H + 1, :], 1.0)
            nc.vector.scalar_tensor_tensor(
                out=LH[0:H, :], in0=HL[:, 1, :], scalar=SLOPE,
                in1=HL[:, 1, :], op0=ALU.mult, op1=ALU.max)
            psO = psp.tile([N_ACT, NB], f32, tag="psB")
            nc.tensor.matmul(psO, FCT, LH, start=True, stop=True)
            OS = persist.tile([N_ACT, NB], f32)
            nc.vector.tensor_copy(OS, psO)
            nc.sync.dma_start(out=d_outT, in_=OS)

    nc.compile()
    return nc


def kernel(**inputs):
    meta, in_maps = prep_host(**inputs)
    nc = build_program(meta)
    res = bass_utils.run_bass_kernel_spmd(nc, in_maps,
                                          core_ids=list(range(NCORES)))
    B = meta["B"]
    out = np.zeros((B, N_ACT), F32)
    h = np.zeros((2, B, H), F32)
    for c in range(NCORES):
        r = res.results[c]
        out[c::NCORES, :] = r["outT"].T
        h[0, c::NCORES, :] = r["hlast"][0].T
        h[1, c::NCORES, :] = r["hlast"][1].T
    return out, h
